# revision 31
# baseline (speedup 1.0000x reference)
"""Trainium2 Bass kernel for tied-row MSA attention (nn_Attention_52329881535135).

Strategy (8 NeuronCores, one chip):
  - Shard the MSA row dim r (leading b*r=256) across the 8 cores: 32 rows each.
  - The host pre-transposes and pre-casts x to bf16 tiles xT[r, dt, p, n]
    (d on partitions) so the device only does plain contiguous DMA loads
    (no f32->bf16 cast bounce, no DMA-transpose on the critical path).
  - Phase 1 is split into four head-pair quarters: each quarter projects
    q/k for heads (2g, 2g+1) for all 32 local rows (row pairs stacked into
    the 128-partition contraction), computes the row-tied logits
    dotsT[j, i] = sum_pairs k^T q in a single 16-matmul PSUM accumulation
    chain per (head, j-tile), and launches a 1MB bf16 AllReduce for its two
    heads.  All four AllReduces complete under the shadow of later compute.
  - Softmax for each head pair runs on ACT/DVE as soon as its AllReduce
    lands (exp with folded column-mask bias, column sums via ones-matmul,
    1/sum folded back into the exp tiles), overlapped with the next
    quarter's matmuls.
  - v projections + a single merged attention pass per row pair: attn @ v
    for all 8 heads, then the full output projection (accumulating all four
    hd-tiles in PSUM) and a single f32 store per row — no DRAM accumulate
    round trip.  Emitted in two half-batches of 8 pairs to bound SBUF.

  Mask bookkeeping (has_rows / num_rows / mask_any) is computed on the host
  at call time and folded into the weights / an additive column bias, so the
  device graph only does dense matmuls.
"""

import sys

sys.path.insert(0, "/opt/trn_rl_repo")

import numpy as np

B, R, N, D, H, DH = 1, 256, 512, 256, 8, 64
INNER = H * DH
NCORES = 8
R_LOC = R // NCORES  # 32 rows per core
P = 128
NPT = N // P  # 4 position tiles
NJT = N // P  # 4 j tiles
NDT = D // P  # 2 d tiles
NHT = INNER // P  # 4 hd tiles
PAIRS = R_LOC // 2  # 16 row pairs
HG = 4  # AllReduce chunks (one per head pair)
H_PER = H // HG  # 2 heads per chunk

_graph_cache = {}


def _build(
    separate_xq: bool,
    has_bias: bool = True,
    r_loc: int = R_LOC,
    n_cores: int = NCORES,
    do_finalize: bool = True,
):
    from contextlib import ExitStack

    from concourse import bacc, mybir, tile

    f32 = mybir.dt.float32
    bf16 = mybir.dt.bfloat16
    AF = mybir.ActivationFunctionType
    ALU = mybir.AluOpType

    pairs = r_loc // 2

    nc = bacc.Bacc(
        "TRN2", target_bir_lowering=False, debug=False, num_devices=n_cores
    )

    xt_ext = nc.declare_dram_parameter("xT", [r_loc, NDT, P, N], bf16, isOutput=False)
    if separate_xq:
        xqt_ext = nc.declare_dram_parameter(
            "xqT", [r_loc, NDT, P, N], bf16, isOutput=False
        )
    wq_ext = nc.declare_dram_parameter("Wq", [P, NDT, INNER], bf16, isOutput=False)
    wk_ext = nc.declare_dram_parameter("Wk", [P, NDT, INNER], bf16, isOutput=False)
    wv_ext = nc.declare_dram_parameter("Wv", [P, NDT, INNER], bf16, isOutput=False)
    wo_ext = nc.declare_dram_parameter("Wo", [P, NHT, D], bf16, isOutput=False)
    bo_ext = nc.declare_dram_parameter("bo", [D], f32, isOutput=False)
    jb_ext = nc.declare_dram_parameter("jbias", [NJT, P], f32, isOutput=False)
    out_ext = nc.declare_dram_parameter("out", [r_loc, N, D], f32, isOutput=True)

    cc_shape = [P, H_PER, NJT, N]
    out_space = "Shared" if n_cores > 4 else "Local"
    cc_in = [nc.dram_tensor(f"cc_in_{g}", cc_shape, bf16) for g in range(HG)]
    cc_out = [
        nc.dram_tensor(f"cc_out_{g}", cc_shape, bf16, addr_space=out_space)
        for g in range(HG)
    ]

    with tile.TileContext(nc) as tc, ExitStack() as top:
        consts = top.enter_context(tc.tile_pool(name="consts", bufs=1))
        exp_pool = top.enter_context(tc.tile_pool(name="expp", bufs=1))
        dl_pool = top.enter_context(tc.tile_pool(name="dlp", bufs=2))
        rs_pool = top.enter_context(tc.tile_pool(name="rsp", bufs=2))
        sm_pool = top.enter_context(tc.tile_pool(name="smp", bufs=2))
        smpsum = top.enter_context(tc.tile_pool(name="smpsum", bufs=1, space="PSUM"))

        # --- constants / weights (already bf16 + pre-rearranged on host);
        # wv/wo/jb ride the ACT HWDGE ring so wq/wk/xT lead the SP ring ---
        wv_sb = consts.tile([P, NDT, INNER], bf16, name="wv_sb")
        nc.scalar.dma_start(wv_sb[:], wv_ext[:])
        wo_sb = consts.tile([P, NHT, D], bf16, name="wo_sb")
        nc.scalar.dma_start(wo_sb[:], wo_ext[:])

        ones_col = consts.tile([P, 1], bf16, name="ones_col")
        nc.any.memset(ones_col, 1.0)
        ones_row = consts.tile([1, P], bf16, name="ones_row")
        nc.any.memset(ones_row, 1.0)
        jb_sb = consts.tile([P, NJT], f32, name="jb_sb")
        nc.scalar.dma_start(jb_sb[:], jb_ext.rearrange("t p -> p t"))
        if has_bias:
            ones_row_f = consts.tile([1, P], f32, name="ones_row_f")
            nc.any.memset(ones_row_f, 1.0)
            bo_sb = consts.tile([1, D], f32, name="bo_sb")
            nc.sync.dma_start(bo_sb[:], bo_ext[None, :])
            bo_bcast = consts.tile([P, D], f32, name="bo_bcast")
            with tc.tile_pool(name="initpsum", bufs=1, space="PSUM") as initp:
                bp0 = initp.tile([P, D], f32, name="bp0")
                nc.tensor.matmul(
                    bp0[:], ones_row_f[:], bo_sb[:], start=True, stop=True
                )
                nc.any.tensor_copy(out=bo_bcast[:], in_=bp0[:])

        exp_sb = exp_pool.tile([P, H, NJT, N], bf16, name="exp_sb")

        from concourse.tile_rust import add_dep_helper

        # ---- softmax, two emission halves so neither the ACT queue nor the
        # PE queue ever head-of-line blocks on an un-landed AllReduce ----
        def softmax_load_exp(h, after=None):
            g, hh = h // H_PER, h % H_PER
            dl = dl_pool.tile([P, NJT, N], bf16, tag="dl", name=f"dl{h}")
            tr = nc.sync.dma_start(dl[:], cc_out[g][:, hh, :, :])
            if after is not None:
                add_dep_helper(tr.ins, after, reason="hold exp until AR window")
            for jt in range(NJT):
                nc.scalar.activation(
                    exp_sb[:, h, jt, :],
                    dl[:, jt, :],
                    AF.Exp,
                    bias=jb_sb[:, jt : jt + 1],
                    scale=1.0,
                )

        def softmax_norm(h, after=None):
            sp = smpsum.tile([1, N], f32, tag="sm", name=f"sp{h}")
            for jt in range(NJT):
                mm = nc.tensor.matmul(
                    sp[:],
                    ones_col[:],
                    exp_sb[:, h, jt, :],
                    start=(jt == 0),
                    stop=(jt == NJT - 1),
                )
                # keep the scheduler from slotting the colsum into an earlier
                # PE-idle moment where its exp isn't ready yet at runtime
                if after is not None and jt == 0:
                    add_dep_helper(mm.ins, after, reason="hold colsum")
            s_bf = sm_pool.tile([1, N], bf16, tag="s_bf", name=f"sbf{h}")
            nc.vector.tensor_copy(s_bf[:], sp[:])
            bps = smpsum.tile([P, N], f32, tag="sm", name=f"bps{h}")
            nc.tensor.matmul(bps[:], ones_row[:], s_bf[:], start=True, stop=True)
            rs = rs_pool.tile([P, N], f32, tag="rs", name=f"rs{h}")
            nc.vector.reciprocal_approx_fast(rs[:], bps[:])
            for jt in range(NJT):
                # on GpSimd (otherwise idle): keeps DVE free for evacuations
                nc.gpsimd.tensor_tensor(
                    exp_sb[:, h, jt, :],
                    exp_sb[:, h, jt, :],
                    rs[:],
                    ALU.mult,
                )

        # =========== Scope 1: four head-pair quarters of q/k + dots ==========
        with ExitStack() as sc1:
            wqk_pool = sc1.enter_context(tc.tile_pool(name="wqk", bufs=1))
            xt_pool = sc1.enter_context(tc.tile_pool(name="xt", bufs=1))
            ccsb_pool = sc1.enter_context(tc.tile_pool(name="ccsb", bufs=4))

            # load order minimizes time-to-first-matmul: wq, rows 0-1, wk,
            # then the remaining resident x^T rows (one tile per row for
            # precise per-row deps + prefetch)
            wq_sb = wqk_pool.tile([P, NDT, INNER], bf16, name="wq_sb")
            nc.sync.dma_start(wq_sb[:], wq_ext[:])
            wk_sb = wqk_pool.tile([P, NDT, INNER], bf16, name="wk_sb")
            xts = []
            xqts = []

            def load_row(r):
                xt = xt_pool.tile([P, NDT, N], bf16, tag=f"xt{r}", name=f"xt{r}")
                nc.sync.dma_start(xt[:], xt_ext[r].rearrange("t p n -> p t n"))
                xts.append(xt)
                if separate_xq:
                    xqt = xt_pool.tile(
                        [P, NDT, N], bf16, tag=f"xqt{r}", name=f"xqt{r}"
                    )
                    nc.sync.dma_start(xqt[:], xqt_ext[r].rearrange("t p n -> p t n"))
                    xqts.append(xqt)
                else:
                    xqts.append(xt)

            load_row(0)
            load_row(1)
            nc.sync.dma_start(wk_sb[:], wk_ext[:])
            for r in range(2, r_loc):
                load_row(r)

            qk_pool = sc1.enter_context(tc.tile_pool(name="qk", bufs=1))
            pp_psum = sc1.enter_context(
                tc.tile_pool(name="pp", bufs=4, space="PSUM")
            )
            dp_psum = sc1.enter_context(
                tc.tile_pool(name="dp", bufs=2, space="PSUM")
            )

            dots_marker = [None] * HG
            for g in range(HG):
                if True:
                    q2 = qk_pool.tile(
                        [P, H_PER, pairs, N], bf16, tag="q2", name=f"q2_{g}"
                    )
                    k2 = qk_pool.tile(
                        [P, H_PER, pairs, N], bf16, tag="k2", name=f"k2_{g}"
                    )
                    for pair in range(pairs):
                        # normalization chain for the heads two quarters back,
                        # mid-quarter: their exps are long done, so the small
                        # colsum matmuls never stall the PE queue
                        if pair == pairs // 2 and g >= 2:
                            softmax_norm(2 * (g - 2), after=proj_marker)
                            softmax_norm(2 * (g - 2) + 1, after=proj_marker)
                        r0 = 2 * pair
                        ecnt = 0
                        for wsb, xpair, dest in (
                            (wq_sb, (xqts[r0], xqts[r0 + 1]), q2),
                            (wk_sb, (xts[r0], xts[r0 + 1]), k2),
                        ):
                            for hh in range(H_PER):
                                h = 2 * g + hh
                                # col-tiled M=64 matmuls: the two row parities
                                # land in partition halves of one PSUM bank
                                # concurrently (distinct col groups), so the
                                # evacuation is a single full-width copy
                                pp = pp_psum.tile([P, N], f32, tag="pp")
                                for dt in range(NDT):
                                    for par in range(2):
                                        nc.tensor.matmul(
                                            pp[64 * par : 64 * par + 64, :],
                                            wsb[:, dt, h * DH : (h + 1) * DH],
                                            xpair[par][:, dt, :],
                                            # has_written clears are per-region:
                                            # each partition half needs its own
                                            # start=True on its first matmul
                                            start=(dt == 0),
                                            stop=(dt == NDT - 1 and par == 1),
                                        )
                                if ecnt % 2 == 0:
                                    pev = nc.vector.tensor_copy(
                                        dest[:, hh, pair, :], pp[:]
                                    )
                                else:
                                    pev = nc.scalar.copy(
                                        dest[:, hh, pair, :], pp[:]
                                    )
                                ecnt += 1
                                if pair == 6:
                                    proj_marker = pev.ins
                    # tied logits: one 16-matmul accumulation chain per (h, jt)
                    for hh in range(H_PER):
                        for jt in range(NJT):
                            dp = dp_psum.tile([P, N], f32, tag="dp")
                            for pq in range(pairs):
                                nc.tensor.matmul(
                                    dp[:],
                                    k2[:, hh, pq, jt * P : (jt + 1) * P],
                                    q2[:, hh, pq, :],
                                    start=(pq == 0),
                                    stop=(pq == pairs - 1),
                                )
                            # dots evacs all ride DVE: ACT must stay clear of
                            # the dp-bank recycle path, because the AR-gated
                            # exps sit in the ACT queue and can block it for
                            # the AR's tail when an AllReduce runs long
                            cc_t = ccsb_pool.tile([P, N], bf16, tag="ccsb")
                            ev = nc.vector.tensor_copy(cc_t[:], dp[:])
                            nc.sync.dma_start(cc_in[g][:, hh, jt, :], cc_t[:])
                            # release the exps off the FIRST dots evac: ACT is
                            # idle during the dots phase (all evacs on DVE), so
                            # even a late AllReduce blocks nothing there
                            if hh == 0 and jt == 0:
                                dots_marker[g] = ev.ins
                nc.gpsimd.collective_compute(
                    "AllReduce",
                    ALU.add,
                    replica_groups=[list(range(n_cores))],
                    ins=[cc_in[g][:]],
                    outs=[cc_out[g][:]],
                )
                # exp for the previous quarter's heads: its AR lands while this
                # quarter computes; the dep on this quarter's last dots-evac
                # keeps the scheduler from hoisting the exps ahead of
                # independent evacuations in the strict-FIFO ACT queue
                if g >= 1:
                    softmax_load_exp(2 * (g - 1), after=dots_marker[g])
                    softmax_load_exp(2 * (g - 1) + 1, after=dots_marker[g])
            softmax_load_exp(4, after=dots_marker[3])
            softmax_load_exp(5, after=dots_marker[3])

        # ===== Scope 2: v projections + merged attn/out pass per row pair ====
        with ExitStack() as sc2:
            xt2_pool = sc2.enter_context(tc.tile_pool(name="xt2", bufs=4))
            v2_pool = sc2.enter_context(tc.tile_pool(name="v2p", bufs=13))
            vpsum = sc2.enter_context(tc.tile_pool(name="vpsum", bufs=3, space="PSUM"))
            out2_pool = sc2.enter_context(tc.tile_pool(name="o2p", bufs=3))
            yrow_pool = sc2.enter_context(tc.tile_pool(name="yrow", bufs=3))
            ap_psum = sc2.enter_context(tc.tile_pool(name="ap", bufs=2, space="PSUM"))
            yp_psum = sc2.enter_context(tc.tile_pool(name="yp", bufs=2, space="PSUM"))

            v2s = {}

            def emit_v(pair):
                v2 = v2_pool.tile(
                    [P, NJT, H, 2, DH], bf16, tag="v2", name=f"v2_{pair}"
                )
                ev = None
                for parity in range(2):
                    r = 2 * pair + parity
                    xt = xt2_pool.tile([P, NDT, N], bf16, tag="xt2", name=f"x2_{r}")
                    nc.sync.dma_start(xt[:], xt_ext[r].rearrange("t p n -> p t n"))
                    for pt in range(NPT):
                        vp = vpsum.tile([P, INNER], f32, tag="vp")
                        for dt in range(NDT):
                            nc.tensor.matmul(
                                vp[:],
                                xt[:, dt, pt * P : (pt + 1) * P],
                                wv_sb[:, dt, :],
                                start=(dt == 0),
                                stop=(dt == NDT - 1),
                            )
                        # alternate evac engines: a single engine cannot keep
                        # pace with the v-projection matmuls
                        evac = nc.vector.tensor_copy if pt % 2 else nc.scalar.copy
                        ev = evac(
                            v2[:, pt, :, parity, :],
                            vp.rearrange("p (h d) -> p h d", h=H),
                        )
                v2s[pair] = v2
                return ev.ins

            def attn_pair(pair):
                v2 = v2s.pop(pair)
                out2 = [
                    out2_pool.tile(
                        [P, NHT, N], bf16, tag=f"o2_{par}", name=f"o2_{par}_{pair}"
                    )
                    for par in range(2)
                ]
                # col-tiled M=64 matmuls: both heads of an hd-tile land in
                # partition halves of one PSUM bank (concurrent col groups),
                # giving a single full-width evacuation per (parity, hd-tile)
                for par in range(2):
                    for t2 in range(NHT):
                        ap = ap_psum.tile([P, N], f32, tag="ap")
                        for jt in range(NJT):
                            for sub in range(2):
                                h = 2 * t2 + sub
                                nc.tensor.matmul(
                                    ap[64 * sub : 64 * sub + 64, :],
                                    v2[:, jt, h, par, :],
                                    exp_sb[:, h, jt, :],
                                    # per-region has_written: start on each
                                    # half's first matmul
                                    start=(jt == 0),
                                    stop=(jt == NJT - 1 and sub == 1),
                                )
                        if (par + t2) % 2 == 0:
                            nc.vector.tensor_copy(out2[par][:, t2, :], ap[:])
                        else:
                            nc.scalar.copy(out2[par][:, t2, :], ap[:])
                last = pair >= pairs - 2
                for par in range(2):
                    r = 2 * pair + par
                    yrow = yrow_pool.tile([P, NPT, D], f32, tag="yrow")
                    dst = out_ext[r].rearrange("(po pi) e -> pi po e", pi=P)
                    for it in range(NPT):
                        yp = yp_psum.tile([P, D], f32, tag="yp")
                        for t2 in range(NHT):
                            nc.tensor.matmul(
                                yp[:],
                                out2[par][:, t2, it * P : (it + 1) * P],
                                wo_sb[:, t2, :],
                                start=(t2 == 0),
                                stop=(t2 == NHT - 1),
                            )
                        if has_bias:
                            nc.vector.tensor_add(
                                out=yrow[:, it, :], in0=yp[:], in1=bo_bcast[:]
                            )
                        elif it % 2 == 0:
                            nc.vector.tensor_copy(yrow[:, it, :], yp[:])
                        else:
                            nc.scalar.copy(yrow[:, it, :], yp[:])
                        # final pairs: store per position-tile so the last DMA
                        # overlaps the out-projection tail instead of
                        # following it
                        if last:
                            nc.sync.dma_start(dst[:, it, :], yrow[:, it, :])
                    if not last:
                        nc.gpsimd.dma_start(dst, yrow[:])

            # 12 v-projections up front (the last AR + exp h6/h7 land under
            # them), then attn pairs with the remaining v-projections woven in
            lead = 12
            for pair in range(lead):
                vm = emit_v(pair)
                if pair == 1:
                    softmax_norm(4, after=vm)
                elif pair == 3:
                    softmax_norm(5, after=vm)
                elif pair == 7:
                    softmax_load_exp(6, after=vm)
                    softmax_load_exp(7, after=vm)
                elif pair == 9:
                    softmax_norm(6, after=vm)
                elif pair == 10:
                    softmax_norm(7, after=vm)
            for i, pair in enumerate(range(lead, pairs)):
                attn_pair(i)
                emit_v(pair)
            for i in range(pairs - lead, pairs):
                attn_pair(i)

    if do_finalize:
        nc.finalize()
    return nc


def _get_graph(separate_xq: bool, has_bias: bool):
    key = (separate_xq, has_bias)
    if key not in _graph_cache:
        _graph_cache[key] = _build(separate_xq, has_bias)
    return _graph_cache[key]


def _prepare(x, mask, Wq, Wk, Wv, Wo, bo, tie_attn_dim):
    """Host-side prep: mask bookkeeping, weight folding, x transpose+cast,
    sharded in_maps."""
    import ml_dtypes

    bf = ml_dtypes.bfloat16

    x = np.asarray(x, dtype=np.float32)
    mask = np.asarray(mask).astype(bool)
    Wq = np.asarray(Wq, dtype=np.float32)
    Wk = np.asarray(Wk, dtype=np.float32)
    Wv = np.asarray(Wv, dtype=np.float32)
    Wo = np.asarray(Wo, dtype=np.float32)
    bo = np.ascontiguousarray(np.asarray(bo, dtype=np.float32))
    r = int(tie_attn_dim)
    assert x.shape == (B * R, N, D) and r == R, (x.shape, r)

    m = mask.reshape(B, R, N)
    has_rows = m.any(axis=-1)[0]  # [R]
    num_rows = max(int(has_rows.sum()), 1)
    col_valid = m.any(axis=1)[0]  # [N]

    scale = (DH ** -0.5) * (num_rows ** -0.5)
    Wq_eff = Wq * np.float32(scale)

    def prep_w(w):  # [D, INNER] -> [P, NDT, INNER] bf16
        return np.ascontiguousarray(
            w.reshape(NDT, P, -1).transpose(1, 0, 2).astype(bf)
        )

    wq_b = prep_w(Wq_eff)
    wk_b = prep_w(Wk)
    wv_b = prep_w(Wv)
    wo_b = np.ascontiguousarray(
        Wo.reshape(NHT, P, D).transpose(1, 0, 2).astype(bf)
    )

    jbias = np.where(col_valid, 0.0, -1e30).astype(np.float32)
    jbias = np.ascontiguousarray(jbias.reshape(NJT, P))

    has_bias = bool(np.any(bo != 0.0))
    separate_xq = not bool(has_rows.all())

    in_maps = []
    for c in range(NCORES):
        xs = x[c * R_LOC : (c + 1) * R_LOC]  # [r_loc, N, D]
        xT = np.ascontiguousarray(
            xs.transpose(0, 2, 1).reshape(R_LOC, NDT, P, N).astype(bf)
        )
        im = {
            "xT": xT,
            "Wq": wq_b,
            "Wk": wk_b,
            "Wv": wv_b,
            "Wo": wo_b,
            "bo": bo,
            "jbias": jbias,
        }
        if separate_xq:
            hr = has_rows[c * R_LOC : (c + 1) * R_LOC].astype(np.float32)
            xq = xs * hr[:, None, None]
            im["xqT"] = np.ascontiguousarray(
                xq.transpose(0, 2, 1).reshape(R_LOC, NDT, P, N).astype(bf)
            )
        in_maps.append(im)
    return separate_xq, has_bias, in_maps


def _warmup(nc, in_maps):
    """Run the NEFF untraced to pull the device out of its idle power state
    (HAM/GPIO throttle) so the subsequent measured run is representative."""
    import os

    from concourse.bass_utils import run_bass_kernel_spmd

    prev = os.environ.get("BASS_NEVER_TRACE")
    os.environ["BASS_NEVER_TRACE"] = "1"
    try:
        for _ in range(2):
            run_bass_kernel_spmd(nc, in_maps, list(range(NCORES)))
    except Exception:
        pass  # warmup is best-effort
    finally:
        if prev is None:
            os.environ.pop("BASS_NEVER_TRACE", None)
        else:
            os.environ["BASS_NEVER_TRACE"] = prev


def kernel(x, mask, Wq, Wk, Wv, Wo, bo, tie_attn_dim):
    from concourse.bass_utils import run_bass_kernel_spmd

    separate_xq, has_bias, in_maps = _prepare(
        x, mask, Wq, Wk, Wv, Wo, bo, tie_attn_dim
    )
    nc = _get_graph(separate_xq, has_bias)
    _warmup(nc, in_maps)
    res = run_bass_kernel_spmd(nc, in_maps, list(range(NCORES)))
    out = np.concatenate([res.results[c]["out"] for c in range(NCORES)], axis=0)
    return out.astype(np.float32)


def _install_ntff_hook():
    """The agent image's antenv lacks axon_hooks; recreate it so trace=True
    can drive NTFF profiling through libaxon_pjrt.so (see trn_boot.py)."""
    try:
        from antenv import axon_hooks  # noqa: F401

        return
    except ImportError:
        pass
    import types

    import antenv

    mod = types.ModuleType("antenv.axon_hooks")
    holder = {}
    mod.set_axon_ntff_profile_hook = lambda h: holder.__setitem__("h", h)
    mod.get_axon_ntff_profile_hook = lambda: holder.get("h")
    sys.modules["antenv.axon_hooks"] = mod
    antenv.axon_hooks = mod
    if "/root/.axon_site" not in sys.path:
        sys.path.insert(0, "/root/.axon_site")
    from trn_agent_boot.trn_boot import _ntff_profile_via_ctypes

    mod.set_axon_ntff_profile_hook(
        _ntff_profile_via_ctypes("/opt/axon/libaxon_pjrt.so")
    )


def bench(inputs):
    """Run with neuron-profile tracing; returns (BassKernelResults, output)."""
    from concourse.bass_utils import run_bass_kernel_spmd

    _install_ntff_hook()
    separate_xq, has_bias, in_maps = _prepare(**inputs)
    nc = _get_graph(separate_xq, has_bias)
    _warmup(nc, in_maps)
    res = run_bass_kernel_spmd(nc, in_maps, list(range(NCORES)), trace=True)
    out = np.concatenate([res.results[c]["out"] for c in range(NCORES)], axis=0)
    return res, out.astype(np.float32)


# revision 32
# speedup vs baseline: 1.1680x; 1.1680x over previous
"""Trainium2 Bass kernel for tied-row MSA attention (nn_Attention_52329881535135).

Strategy (8 NeuronCores, one chip):
  - Shard the MSA row dim r (leading b*r=256) across the 8 cores: 32 rows each.
  - The host pre-transposes and pre-casts x to bf16 tiles xT[r, dt, p, n]
    (d on partitions) so the device only does plain contiguous DMA loads
    (no f32->bf16 cast bounce, no DMA-transpose on the critical path).
  - Phase 1 is split into four head-pair quarters: each quarter projects
    q/k for heads (2g, 2g+1) for all 32 local rows (row pairs stacked into
    the 128-partition contraction), computes the row-tied logits
    dotsT[j, i] = sum_pairs k^T q in a single 16-matmul PSUM accumulation
    chain per (head, j-tile), and launches a 1MB bf16 AllReduce for its two
    heads.  All four AllReduces complete under the shadow of later compute.
  - Softmax for each head pair runs on ACT/DVE as soon as its AllReduce
    lands (exp with folded column-mask bias, column sums via ones-matmul,
    1/sum folded back into the exp tiles), overlapped with the next
    quarter's matmuls.
  - v projections + a single merged attention pass per row pair: attn @ v
    for all 8 heads, then the full output projection (accumulating all four
    hd-tiles in PSUM) and a single f32 store per row — no DRAM accumulate
    round trip.  Emitted in two half-batches of 8 pairs to bound SBUF.

  Mask bookkeeping (has_rows / num_rows / mask_any) is computed on the host
  at call time and folded into the weights / an additive column bias, so the
  device graph only does dense matmuls.
"""

import sys

sys.path.insert(0, "/opt/trn_rl_repo")

import numpy as np

B, R, N, D, H, DH = 1, 256, 512, 256, 8, 64
INNER = H * DH
NCORES = 8
R_LOC = R // NCORES  # 32 rows per core
P = 128
NPT = N // P  # 4 position tiles
NJT = N // P  # 4 j tiles
NDT = D // P  # 2 d tiles
NHT = INNER // P  # 4 hd tiles
PAIRS = R_LOC // 2  # 16 row pairs
HG = 4  # AllReduce chunks (one per head pair)
H_PER = H // HG  # 2 heads per chunk

_graph_cache = {}


def _build(
    separate_xq: bool,
    has_bias: bool = True,
    r_loc: int = R_LOC,
    n_cores: int = NCORES,
    do_finalize: bool = True,
):
    from contextlib import ExitStack

    from concourse import bacc, mybir, tile

    f32 = mybir.dt.float32
    bf16 = mybir.dt.bfloat16
    AF = mybir.ActivationFunctionType
    ALU = mybir.AluOpType

    pairs = r_loc // 2

    nc = bacc.Bacc(
        "TRN2", target_bir_lowering=False, debug=False, num_devices=n_cores
    )

    xt_ext = nc.declare_dram_parameter("xT", [r_loc, NDT, P, N], bf16, isOutput=False)
    if separate_xq:
        xqt_ext = nc.declare_dram_parameter(
            "xqT", [r_loc, NDT, P, N], bf16, isOutput=False
        )
    wq_ext = nc.declare_dram_parameter("Wq", [P, NDT, INNER], bf16, isOutput=False)
    wk_ext = nc.declare_dram_parameter("Wk", [P, NDT, INNER], bf16, isOutput=False)
    wv_ext = nc.declare_dram_parameter("Wv", [P, NDT, INNER], bf16, isOutput=False)
    wo_ext = nc.declare_dram_parameter("Wo", [P, NHT, D], bf16, isOutput=False)
    bo_ext = nc.declare_dram_parameter("bo", [D], f32, isOutput=False)
    jb_ext = nc.declare_dram_parameter("jbias", [NJT, P], f32, isOutput=False)
    out_ext = nc.declare_dram_parameter("out", [r_loc, N, D], f32, isOutput=True)

    cc_shape = [P, H_PER, NJT, N]
    out_space = "Shared" if n_cores > 4 else "Local"
    cc_in = [nc.dram_tensor(f"cc_in_{g}", cc_shape, bf16) for g in range(HG)]
    cc_out = [
        nc.dram_tensor(f"cc_out_{g}", cc_shape, bf16, addr_space=out_space)
        for g in range(HG)
    ]

    with tile.TileContext(nc) as tc, ExitStack() as top:
        consts = top.enter_context(tc.tile_pool(name="consts", bufs=1))
        exp_pool = top.enter_context(tc.tile_pool(name="expp", bufs=1))
        dl_pool = top.enter_context(tc.tile_pool(name="dlp", bufs=2))
        rs_pool = top.enter_context(tc.tile_pool(name="rsp", bufs=2))
        sm_pool = top.enter_context(tc.tile_pool(name="smp", bufs=2))
        smpsum = top.enter_context(tc.tile_pool(name="smpsum", bufs=1, space="PSUM"))

        # --- constants / weights (already bf16 + pre-rearranged on host);
        # wv/wo/jb ride the ACT HWDGE ring so wq/wk/xT lead the SP ring ---
        wv_sb = consts.tile([P, NDT, INNER], bf16, name="wv_sb")
        nc.scalar.dma_start(wv_sb[:], wv_ext[:])
        wo_sb = consts.tile([P, NHT, D], bf16, name="wo_sb")
        nc.scalar.dma_start(wo_sb[:], wo_ext[:])

        ones_col = consts.tile([P, 1], bf16, name="ones_col")
        nc.any.memset(ones_col, 1.0)
        ones_row = consts.tile([1, P], bf16, name="ones_row")
        nc.any.memset(ones_row, 1.0)
        jb_sb = consts.tile([P, NJT], f32, name="jb_sb")
        nc.scalar.dma_start(jb_sb[:], jb_ext.rearrange("t p -> p t"))
        if has_bias:
            ones_row_f = consts.tile([1, P], f32, name="ones_row_f")
            nc.any.memset(ones_row_f, 1.0)
            bo_sb = consts.tile([1, D], f32, name="bo_sb")
            nc.sync.dma_start(bo_sb[:], bo_ext[None, :])
            bo_bcast = consts.tile([P, D], f32, name="bo_bcast")
            with tc.tile_pool(name="initpsum", bufs=1, space="PSUM") as initp:
                bp0 = initp.tile([P, D], f32, name="bp0")
                nc.tensor.matmul(
                    bp0[:], ones_row_f[:], bo_sb[:], start=True, stop=True
                )
                nc.any.tensor_copy(out=bo_bcast[:], in_=bp0[:])

        exp_sb = exp_pool.tile([P, H, NJT, N], bf16, name="exp_sb")

        from concourse.tile_rust import add_dep_helper

        # ---- softmax, two emission halves so neither the ACT queue nor the
        # PE queue ever head-of-line blocks on an un-landed AllReduce ----
        def softmax_load_exp(h, after=None):
            g, hh = h // H_PER, h % H_PER
            dl = dl_pool.tile([P, NJT, N], bf16, tag="dl", name=f"dl{h}")
            # ACT HWDGE ring: the SP ring carries cc_in/xT traffic that must
            # never queue behind this AR-gated load
            tr = nc.scalar.dma_start(dl[:], cc_out[g][:, hh, :, :])
            if after is not None:
                add_dep_helper(tr.ins, after, reason="hold exp until AR window")
            for jt in range(NJT):
                nc.scalar.activation(
                    exp_sb[:, h, jt, :],
                    dl[:, jt, :],
                    AF.Exp,
                    bias=jb_sb[:, jt : jt + 1],
                    scale=1.0,
                )

        def softmax_norm(h, after=None):
            sp = smpsum.tile([1, N], f32, tag="sm", name=f"sp{h}")
            for jt in range(NJT):
                mm = nc.tensor.matmul(
                    sp[:],
                    ones_col[:],
                    exp_sb[:, h, jt, :],
                    start=(jt == 0),
                    stop=(jt == NJT - 1),
                )
                # keep the scheduler from slotting the colsum into an earlier
                # PE-idle moment where its exp isn't ready yet at runtime
                if after is not None and jt == 0:
                    add_dep_helper(mm.ins, after, reason="hold colsum")
            s_bf = sm_pool.tile([1, N], bf16, tag="s_bf", name=f"sbf{h}")
            nc.vector.tensor_copy(s_bf[:], sp[:])
            bps = smpsum.tile([P, N], f32, tag="sm", name=f"bps{h}")
            nc.tensor.matmul(bps[:], ones_row[:], s_bf[:], start=True, stop=True)
            rs = rs_pool.tile([P, N], f32, tag="rs", name=f"rs{h}")
            nc.vector.reciprocal_approx_fast(rs[:], bps[:])
            for jt in range(NJT):
                # on GpSimd (otherwise idle): keeps DVE free for evacuations
                nc.gpsimd.tensor_tensor(
                    exp_sb[:, h, jt, :],
                    exp_sb[:, h, jt, :],
                    rs[:],
                    ALU.mult,
                )

        # =========== Scope 1: four head-pair quarters of q/k + dots ==========
        with ExitStack() as sc1:
            wqk_pool = sc1.enter_context(tc.tile_pool(name="wqk", bufs=1))
            xt_pool = sc1.enter_context(tc.tile_pool(name="xt", bufs=1))
            ccsb_pool = sc1.enter_context(tc.tile_pool(name="ccsb", bufs=4))

            # load order minimizes time-to-first-matmul: wq, rows 0-1, wk,
            # then the remaining resident x^T rows (one tile per row for
            # precise per-row deps + prefetch)
            wq_sb = wqk_pool.tile([P, NDT, INNER], bf16, name="wq_sb")
            nc.sync.dma_start(wq_sb[:], wq_ext[:])
            wk_sb = wqk_pool.tile([P, NDT, INNER], bf16, name="wk_sb")
            xts = []
            xqts = []

            def load_row(r):
                xt = xt_pool.tile([P, NDT, N], bf16, tag=f"xt{r}", name=f"xt{r}")
                nc.sync.dma_start(xt[:], xt_ext[r].rearrange("t p n -> p t n"))
                xts.append(xt)
                if separate_xq:
                    xqt = xt_pool.tile(
                        [P, NDT, N], bf16, tag=f"xqt{r}", name=f"xqt{r}"
                    )
                    nc.sync.dma_start(xqt[:], xqt_ext[r].rearrange("t p n -> p t n"))
                    xqts.append(xqt)
                else:
                    xqts.append(xt)

            load_row(0)
            load_row(1)
            nc.sync.dma_start(wk_sb[:], wk_ext[:])
            for r in range(2, r_loc):
                load_row(r)

            qk_pool = sc1.enter_context(tc.tile_pool(name="qk", bufs=1))
            pp_psum = sc1.enter_context(
                tc.tile_pool(name="pp", bufs=4, space="PSUM")
            )
            dp_psum = sc1.enter_context(
                tc.tile_pool(name="dp", bufs=2, space="PSUM")
            )

            dots_marker = [None] * HG
            for g in range(HG):
                if True:
                    q2 = qk_pool.tile(
                        [P, H_PER, pairs, N], bf16, tag="q2", name=f"q2_{g}"
                    )
                    k2 = qk_pool.tile(
                        [P, H_PER, pairs, N], bf16, tag="k2", name=f"k2_{g}"
                    )
                    for pair in range(pairs):
                        # normalization chain for the heads two quarters back,
                        # mid-quarter: their exps are long done, so the small
                        # colsum matmuls never stall the PE queue
                        if pair == pairs // 2 and g >= 2:
                            softmax_norm(2 * (g - 2), after=proj_marker)
                            softmax_norm(2 * (g - 2) + 1, after=proj_marker)
                        r0 = 2 * pair
                        ecnt = 0
                        for wsb, xpair, dest in (
                            (wq_sb, (xqts[r0], xqts[r0 + 1]), q2),
                            (wk_sb, (xts[r0], xts[r0 + 1]), k2),
                        ):
                            for hh in range(H_PER):
                                h = 2 * g + hh
                                # col-tiled M=64 matmuls: the two row parities
                                # land in partition halves of one PSUM bank
                                # concurrently (distinct col groups), so the
                                # evacuation is a single full-width copy
                                pp = pp_psum.tile([P, N], f32, tag="pp")
                                for dt in range(NDT):
                                    for par in range(2):
                                        nc.tensor.matmul(
                                            pp[64 * par : 64 * par + 64, :],
                                            wsb[:, dt, h * DH : (h + 1) * DH],
                                            xpair[par][:, dt, :],
                                            # has_written clears are per-region:
                                            # each partition half needs its own
                                            # start=True on its first matmul
                                            start=(dt == 0),
                                            stop=(dt == NDT - 1 and par == 1),
                                        )
                                if ecnt % 2 == 0:
                                    pev = nc.vector.tensor_copy(
                                        dest[:, hh, pair, :], pp[:]
                                    )
                                else:
                                    pev = nc.scalar.copy(
                                        dest[:, hh, pair, :], pp[:]
                                    )
                                ecnt += 1
                                if pair == 6:
                                    proj_marker = pev.ins
                    # tied logits: one 16-matmul accumulation chain per (h, jt)
                    for hh in range(H_PER):
                        for jt in range(NJT):
                            dp = dp_psum.tile([P, N], f32, tag="dp")
                            for pq in range(pairs):
                                nc.tensor.matmul(
                                    dp[:],
                                    k2[:, hh, pq, jt * P : (jt + 1) * P],
                                    q2[:, hh, pq, :],
                                    start=(pq == 0),
                                    stop=(pq == pairs - 1),
                                )
                            # dots evacs all ride DVE: ACT must stay clear of
                            # the dp-bank recycle path, because the AR-gated
                            # exps sit in the ACT queue and can block it for
                            # the AR's tail when an AllReduce runs long
                            cc_t = ccsb_pool.tile([P, N], bf16, tag="ccsb")
                            ev = nc.vector.tensor_copy(cc_t[:], dp[:])
                            nc.sync.dma_start(cc_in[g][:, hh, jt, :], cc_t[:])
                            # release the exps off the FIRST dots evac: ACT is
                            # idle during the dots phase (all evacs on DVE), so
                            # even a late AllReduce blocks nothing there
                            if hh == 0 and jt == 0:
                                dots_marker[g] = ev.ins
                nc.gpsimd.collective_compute(
                    "AllReduce",
                    ALU.add,
                    replica_groups=[list(range(n_cores))],
                    ins=[cc_in[g][:]],
                    outs=[cc_out[g][:]],
                )
                # exp for the previous quarter's heads: its AR lands while this
                # quarter computes; the dep on this quarter's last dots-evac
                # keeps the scheduler from hoisting the exps ahead of
                # independent evacuations in the strict-FIFO ACT queue
                if g >= 1:
                    softmax_load_exp(2 * (g - 1), after=dots_marker[g])
                    softmax_load_exp(2 * (g - 1) + 1, after=dots_marker[g])
            softmax_load_exp(4, after=dots_marker[3])
            softmax_load_exp(5, after=dots_marker[3])

        # ===== Scope 2: v projections + merged attn/out pass per row pair ====
        with ExitStack() as sc2:
            xt2_pool = sc2.enter_context(tc.tile_pool(name="xt2", bufs=4))
            v2_pool = sc2.enter_context(tc.tile_pool(name="v2p", bufs=13))
            vpsum = sc2.enter_context(tc.tile_pool(name="vpsum", bufs=3, space="PSUM"))
            out2_pool = sc2.enter_context(tc.tile_pool(name="o2p", bufs=3))
            yrow_pool = sc2.enter_context(tc.tile_pool(name="yrow", bufs=3))
            ap_psum = sc2.enter_context(tc.tile_pool(name="ap", bufs=2, space="PSUM"))
            yp_psum = sc2.enter_context(tc.tile_pool(name="yp", bufs=2, space="PSUM"))

            v2s = {}

            def emit_v(pair):
                v2 = v2_pool.tile(
                    [P, NJT, H, 2, DH], bf16, tag="v2", name=f"v2_{pair}"
                )
                ev = None
                for parity in range(2):
                    r = 2 * pair + parity
                    xt = xt2_pool.tile([P, NDT, N], bf16, tag="xt2", name=f"x2_{r}")
                    nc.sync.dma_start(xt[:], xt_ext[r].rearrange("t p n -> p t n"))
                    for pt in range(NPT):
                        vp = vpsum.tile([P, INNER], f32, tag="vp")
                        for dt in range(NDT):
                            nc.tensor.matmul(
                                vp[:],
                                xt[:, dt, pt * P : (pt + 1) * P],
                                wv_sb[:, dt, :],
                                start=(dt == 0),
                                stop=(dt == NDT - 1),
                            )
                        # alternate evac engines: a single engine cannot keep
                        # pace with the v-projection matmuls
                        evac = nc.vector.tensor_copy if pt % 2 else nc.scalar.copy
                        ev = evac(
                            v2[:, pt, :, parity, :],
                            vp.rearrange("p (h d) -> p h d", h=H),
                        )
                v2s[pair] = v2
                return ev.ins

            def attn_pair(pair):
                v2 = v2s.pop(pair)
                out2 = [
                    out2_pool.tile(
                        [P, NHT, N], bf16, tag=f"o2_{par}", name=f"o2_{par}_{pair}"
                    )
                    for par in range(2)
                ]
                # col-tiled M=64 matmuls: both heads of an hd-tile land in
                # partition halves of one PSUM bank (concurrent col groups),
                # giving a single full-width evacuation per (parity, hd-tile)
                for par in range(2):
                    for t2 in range(NHT):
                        ap = ap_psum.tile([P, N], f32, tag="ap")
                        for jt in range(NJT):
                            for sub in range(2):
                                h = 2 * t2 + sub
                                nc.tensor.matmul(
                                    ap[64 * sub : 64 * sub + 64, :],
                                    v2[:, jt, h, par, :],
                                    exp_sb[:, h, jt, :],
                                    # per-region has_written: start on each
                                    # half's first matmul
                                    start=(jt == 0),
                                    stop=(jt == NJT - 1 and sub == 1),
                                )
                        if (par + t2) % 2 == 0:
                            nc.vector.tensor_copy(out2[par][:, t2, :], ap[:])
                        else:
                            nc.scalar.copy(out2[par][:, t2, :], ap[:])
                last = pair >= pairs - 2
                for par in range(2):
                    r = 2 * pair + par
                    yrow = yrow_pool.tile([P, NPT, D], f32, tag="yrow")
                    dst = out_ext[r].rearrange("(po pi) e -> pi po e", pi=P)
                    for it in range(NPT):
                        yp = yp_psum.tile([P, D], f32, tag="yp")
                        for t2 in range(NHT):
                            nc.tensor.matmul(
                                yp[:],
                                out2[par][:, t2, it * P : (it + 1) * P],
                                wo_sb[:, t2, :],
                                start=(t2 == 0),
                                stop=(t2 == NHT - 1),
                            )
                        if has_bias:
                            nc.vector.tensor_add(
                                out=yrow[:, it, :], in0=yp[:], in1=bo_bcast[:]
                            )
                        elif it % 2 == 0:
                            nc.vector.tensor_copy(yrow[:, it, :], yp[:])
                        else:
                            nc.scalar.copy(yrow[:, it, :], yp[:])
                        # final pairs: store per position-tile so the last DMA
                        # overlaps the out-projection tail instead of
                        # following it
                        if last:
                            nc.sync.dma_start(dst[:, it, :], yrow[:, it, :])
                    if not last:
                        nc.gpsimd.dma_start(dst, yrow[:])

            # 12 v-projections up front (the last AR + exp h6/h7 land under
            # them), then attn pairs with the remaining v-projections woven in
            lead = 12
            for pair in range(lead):
                vm = emit_v(pair)
                if pair == 1:
                    softmax_norm(4, after=vm)
                elif pair == 3:
                    softmax_norm(5, after=vm)
                elif pair == 7:
                    softmax_load_exp(6, after=vm)
                    softmax_load_exp(7, after=vm)
                elif pair == 9:
                    softmax_norm(6, after=vm)
                elif pair == 10:
                    softmax_norm(7, after=vm)
            for i, pair in enumerate(range(lead, pairs)):
                attn_pair(i)
                emit_v(pair)
            for i in range(pairs - lead, pairs):
                attn_pair(i)

    if do_finalize:
        nc.finalize()
    return nc


def _get_graph(separate_xq: bool, has_bias: bool):
    key = (separate_xq, has_bias)
    if key not in _graph_cache:
        _graph_cache[key] = _build(separate_xq, has_bias)
    return _graph_cache[key]


def _prepare(x, mask, Wq, Wk, Wv, Wo, bo, tie_attn_dim):
    """Host-side prep: mask bookkeeping, weight folding, x transpose+cast,
    sharded in_maps."""
    import ml_dtypes

    bf = ml_dtypes.bfloat16

    x = np.asarray(x, dtype=np.float32)
    mask = np.asarray(mask).astype(bool)
    Wq = np.asarray(Wq, dtype=np.float32)
    Wk = np.asarray(Wk, dtype=np.float32)
    Wv = np.asarray(Wv, dtype=np.float32)
    Wo = np.asarray(Wo, dtype=np.float32)
    bo = np.ascontiguousarray(np.asarray(bo, dtype=np.float32))
    r = int(tie_attn_dim)
    assert x.shape == (B * R, N, D) and r == R, (x.shape, r)

    m = mask.reshape(B, R, N)
    has_rows = m.any(axis=-1)[0]  # [R]
    num_rows = max(int(has_rows.sum()), 1)
    col_valid = m.any(axis=1)[0]  # [N]

    scale = (DH ** -0.5) * (num_rows ** -0.5)
    Wq_eff = Wq * np.float32(scale)

    def prep_w(w):  # [D, INNER] -> [P, NDT, INNER] bf16
        return np.ascontiguousarray(
            w.reshape(NDT, P, -1).transpose(1, 0, 2).astype(bf)
        )

    wq_b = prep_w(Wq_eff)
    wk_b = prep_w(Wk)
    wv_b = prep_w(Wv)
    wo_b = np.ascontiguousarray(
        Wo.reshape(NHT, P, D).transpose(1, 0, 2).astype(bf)
    )

    jbias = np.where(col_valid, 0.0, -1e30).astype(np.float32)
    jbias = np.ascontiguousarray(jbias.reshape(NJT, P))

    has_bias = bool(np.any(bo != 0.0))
    separate_xq = not bool(has_rows.all())

    in_maps = []
    for c in range(NCORES):
        xs = x[c * R_LOC : (c + 1) * R_LOC]  # [r_loc, N, D]
        xT = np.ascontiguousarray(
            xs.transpose(0, 2, 1).reshape(R_LOC, NDT, P, N).astype(bf)
        )
        im = {
            "xT": xT,
            "Wq": wq_b,
            "Wk": wk_b,
            "Wv": wv_b,
            "Wo": wo_b,
            "bo": bo,
            "jbias": jbias,
        }
        if separate_xq:
            hr = has_rows[c * R_LOC : (c + 1) * R_LOC].astype(np.float32)
            xq = xs * hr[:, None, None]
            im["xqT"] = np.ascontiguousarray(
                xq.transpose(0, 2, 1).reshape(R_LOC, NDT, P, N).astype(bf)
            )
        in_maps.append(im)
    return separate_xq, has_bias, in_maps


def _warmup(nc, in_maps):
    """Run the NEFF untraced to pull the device out of its idle power state
    (HAM/GPIO throttle) so the subsequent measured run is representative."""
    import os

    from concourse.bass_utils import run_bass_kernel_spmd

    prev = os.environ.get("BASS_NEVER_TRACE")
    os.environ["BASS_NEVER_TRACE"] = "1"
    try:
        for _ in range(2):
            run_bass_kernel_spmd(nc, in_maps, list(range(NCORES)))
    except Exception:
        pass  # warmup is best-effort
    finally:
        if prev is None:
            os.environ.pop("BASS_NEVER_TRACE", None)
        else:
            os.environ["BASS_NEVER_TRACE"] = prev


def kernel(x, mask, Wq, Wk, Wv, Wo, bo, tie_attn_dim):
    from concourse.bass_utils import run_bass_kernel_spmd

    separate_xq, has_bias, in_maps = _prepare(
        x, mask, Wq, Wk, Wv, Wo, bo, tie_attn_dim
    )
    nc = _get_graph(separate_xq, has_bias)
    _warmup(nc, in_maps)
    res = run_bass_kernel_spmd(nc, in_maps, list(range(NCORES)))
    out = np.concatenate([res.results[c]["out"] for c in range(NCORES)], axis=0)
    return out.astype(np.float32)


def _install_ntff_hook():
    """The agent image's antenv lacks axon_hooks; recreate it so trace=True
    can drive NTFF profiling through libaxon_pjrt.so (see trn_boot.py)."""
    try:
        from antenv import axon_hooks  # noqa: F401

        return
    except ImportError:
        pass
    import types

    import antenv

    mod = types.ModuleType("antenv.axon_hooks")
    holder = {}
    mod.set_axon_ntff_profile_hook = lambda h: holder.__setitem__("h", h)
    mod.get_axon_ntff_profile_hook = lambda: holder.get("h")
    sys.modules["antenv.axon_hooks"] = mod
    antenv.axon_hooks = mod
    if "/root/.axon_site" not in sys.path:
        sys.path.insert(0, "/root/.axon_site")
    from trn_agent_boot.trn_boot import _ntff_profile_via_ctypes

    mod.set_axon_ntff_profile_hook(
        _ntff_profile_via_ctypes("/opt/axon/libaxon_pjrt.so")
    )


def bench(inputs):
    """Run with neuron-profile tracing; returns (BassKernelResults, output)."""
    from concourse.bass_utils import run_bass_kernel_spmd

    _install_ntff_hook()
    separate_xq, has_bias, in_maps = _prepare(**inputs)
    nc = _get_graph(separate_xq, has_bias)
    _warmup(nc, in_maps)
    res = run_bass_kernel_spmd(nc, in_maps, list(range(NCORES)), trace=True)
    out = np.concatenate([res.results[c]["out"] for c in range(NCORES)], axis=0)
    return res, out.astype(np.float32)
